# revision 42
# baseline (speedup 1.0000x reference)
"""AOSPredictionLayer — Trainium2 Bass kernel (8 NeuronCores, expert-sharded).

pred[b,n] = <ui_mlp(concat(u,i))[b], relation_mlp[s[b,n]](concat(a,o)[b,n])>
B=512, N=32, R=8, MLP dims 512->512->256->128 (leaky_relu 0.01, zero biases).

MoE routing: core e gets the tokens of relation e (sorted by batch row,
padded to tp); per-relation weights sharded, UI MLP replicated.

Numerics (all biases are zero; scales fold into activations / host):
  - L1 (expert + UI) as fp8e4m3 DoubleRow (0.5 cyc/col, 2x128 contraction
    per instr), 3-term error-compensated: w' = 16*W split hi/lo, x split
    hi/lo on host; wh*xh + wl*xh + wh*xl accumulate in fp32 PSUM.  Recovers
    bf16-level accuracy at 0.75x the bf16 matmul cost.
  - Expert L2/L3 as fp8 DoubleRow 2-term (wh+wl)*h, h1/h2 written directly
    in fp8 by the activations; UI L2/L3 stay bf16 for accuracy headroom.
  - Activation scale chain: h1_stored=lrelu(16*W1x) [scale 1],
    h2_stored=lrelu(psum/16) [1/16], h3_stored=lrelu(psum)=256*h3;
    host divides pred by 256.  Measured rel err ~1.38e-2 on HW.

Dataflow per core, in 512-token chunks (order: 256a, fulls..., 256b, tail):
  - one-hot rows oh_c = (b == iota+128c) on DVE; psg = ui3_tok^T @ oh
    accumulated across the chunk's b-groups in ONE PSUM group (overlapping
    union spans across cores add; wrong-group terms are zero);
    prod = h3 * psg on DVE; pred = ones^T @ prod; copy + DMA out.
  - Software pipeline, period p emits PE work for three chunks at once:
    L1(p+1) | L2+L3(p) | psg/prod/psd/pc(p-1), so the in-order PE stream
    never waits; the last two chunks drain with interleaved chains.
  - PSUM: L1 pairs in [128,2,512] double-bank tiles (fused pair
    activations), L2 pair likewise; acts on the Act engine, h3 too; DVE
    does one-hots/prod/pc; weights+x arrive split across SP/Act/Pool DMA
    queues with UI-critical halves first.
"""
import sys

for _p in ("/opt/trn_rl_repo", "/opt/pypackages"):
    if _p not in sys.path:
        sys.path.append(_p)

import numpy as np
import ml_dtypes

import concourse.tile as tile
from concourse import bacc, mybir
from concourse.masks import make_identity
from concourse.bass_utils import run_bass_kernel_spmd

F32 = mybir.dt.float32
BF16 = mybir.dt.bfloat16
F16 = mybir.dt.float16
E4 = mybir.dt.float8e4

BF16_NP = ml_dtypes.bfloat16
E4_NP = ml_dtypes.float8_e4m3

B = 512
N_TOK = 32
IN1 = 256
HID = [512, 256, 128]
R = 8
N_CORES = 8

D_AO = 2 * IN1                     # 512
DIMS = [D_AO] + HID                # [512, 512, 256, 128]
WSCALE = 16.0
DR = mybir.MatmulPerfMode.DoubleRow
LR = 0.01                          # leaky-relu slope
BOFF = [0, 4, 6]                   # bias col offset per layer in [128,7]


def _chunks(tp):
    """Processing order: first full chunk split half (256a), then fulls,
    then 256b, tiny tail LAST (shortest drain chain)."""
    fulls = []
    s = 0
    while tp - s > 512:
        fulls.append((s, 512))
        s += 512
    tail = (s, tp - s)
    la, lb = fulls[-1]
    return fulls[:-1] + [(la, 256), (la + 256, 256), tail]


def _halves(n):
    if n <= 256:
        return [(0, n)]
    return [(0, 256), (256, n)]


def _build_kernel(tp, pieces):
    """pieces[ci] = list of (group, lo, hi): union spans of b//128 groups
    in chunk ci across all cores (one-hot zeroes wrong-group terms)."""
    nc = bacc.Bacc("TRN2", target_bir_lowering=False, debug=False,
                   num_devices=N_CORES)

    xh_d = nc.dram_tensor("xh", [128, 4, tp], E4, kind="ExternalInput").ap()
    xl_d = nc.dram_tensor("xl", [128, 4, tp], E4, kind="ExternalInput").ap()
    wd = {}
    for nm, shp in (("wh1", [128, 4, 512]), ("wl1", [128, 4, 512]),
                    ("wh2", [128, 4, 256]), ("wl2", [128, 4, 256]),
                    ("wh3", [128, 2, 128]), ("wl3", [128, 2, 128]),
                    ("uiTh", [128, 4, B]), ("uiTl", [128, 4, B]),
                    ("wuh1", [128, 4, 512]), ("wul1", [128, 4, 512])):
        wd[nm] = nc.dram_tensor(nm, shp, E4, kind="ExternalInput").ap()
    wu2_d = nc.dram_tensor("wu2", [128, 4, 256], BF16,
                           kind="ExternalInput").ap()
    wu3_d = nc.dram_tensor("wu3", [128, 2, 128], BF16,
                           kind="ExternalInput").ap()
    bids_d = nc.dram_tensor("bids", [128, tp], F16, kind="ExternalInput").ap()
    iota_d = nc.dram_tensor("iota4", [128, 4], F32, kind="ExternalInput").ap()
    pred_d = nc.dram_tensor("pred", [1, tp], F32, kind="ExternalOutput").ap()

    chunks = _chunks(tp)
    NCH = len(chunks)

    with tile.TileContext(nc) as tc:
        with (
            tc.tile_pool(name="wts", bufs=1) as wts,
            tc.tile_pool(name="xin", bufs=1) as xin,
            tc.tile_pool(name="uip", bufs=1) as uip,
            tc.tile_pool(name="actp", bufs=2) as actp,
            tc.tile_pool(name="finp", bufs=2) as finp,
            tc.tile_pool(name="mmd", bufs=3, space="PSUM") as mmd,
            tc.tile_pool(name="mm1", bufs=2, space="PSUM") as mm1,
        ):
            # ---- tiny PE warm-up (pins pe_busy_start; ~free) ----
            dum = uip.tile([1, 8], BF16, tag="dum")
            nc.vector.memset(dum[:], 0.0)
            psw = mm1.tile([128, 512], F32, tag="mm", name="psw")
            for _ in range(2):
                nc.tensor.matmul(psw[0:1, 0:8], dum[0:1, 0:1], dum[:],
                                 start=True, stop=True)

            # ---- DMAs ----
            # SP queue: UI-phase-critical tensors in half (k-pair) pieces so
            # the first UI matmuls can start ~1.9us in.
            t_uiTh = wts.tile([128, 4, B], E4, tag="uiTh")
            nc.sync.dma_start(t_uiTh[:, 0:2, :], wd["uiTh"][:, 0:2, :])
            t_wul1 = wts.tile([128, 4, 512], E4, tag="wul1")
            nc.sync.dma_start(t_wul1[:, 0:2, :], wd["wul1"][:, 0:2, :])
            nc.sync.dma_start(t_uiTh[:, 2:4, :], wd["uiTh"][:, 2:4, :])
            nc.sync.dma_start(t_wul1[:, 2:4, :], wd["wul1"][:, 2:4, :])
            t_uiTl = wts.tile([128, 4, B], E4, tag="uiTl")
            nc.sync.dma_start(t_uiTl[:], wd["uiTl"])
            t_xl = xin.tile([128, 4, tp], E4, tag="xl")
            t_xh = xin.tile([128, 4, tp], E4, tag="xh")
            # pre-loop chunks (chunks[0], chunks[1]) first, rest later
            c0s, c0n = None, None

            def x_slices():
                (a0, n0), (a1, n1) = chunks[0], chunks[1]
                lo = min(a0, a1)
                hi = max(a0 + n0, a1 + n1)
                if a0 + n0 == a1 or a1 + n1 == a0:
                    return ([(lo, hi)],
                            [(r, s) for r, s in ((0, lo), (hi, tp)) if s > r])
                return ([(a0, a0 + n0), (a1, a1 + n1)],
                        [(r, s) for r, s in ((0, min(a0, a1)),
                                             (min(a0 + n0, a1 + n1),
                                              max(a0, a1)),
                                             (max(a0 + n0, a1 + n1), tp))
                         if s > r])

            first, rest = x_slices()
            for lo, hi in first:
                nc.sync.dma_start(t_xl[:, :, lo:hi], xl_d[:, :, lo:hi])
            for lo, hi in rest:
                nc.sync.dma_start(t_xl[:, :, lo:hi], xl_d[:, :, lo:hi])
            t_bids = wts.tile([128, tp], F16, tag="bids")
            nc.sync.dma_start(t_bids[:], bids_d)

            # Act queue: UI L2/L3 weights + LUT warm-up (table load gets
            # hoisted to the queue head).
            t_wu2 = wts.tile([128, 4, 256], BF16, tag="wu2")
            nc.scalar.dma_start(t_wu2[:], wu2_d)
            t_wu3 = wts.tile([128, 2, 128], BF16, tag="wu3")
            nc.scalar.dma_start(t_wu3[:], wu3_d)
            lut = uip.tile([1, 1], F32, tag="lut")
            nc.vector.memset(lut[:], 0.0)
            nc.scalar.activation(lut[:], lut[:],
                                 mybir.ActivationFunctionType.Lrelu,
                                 bias=0.0, scale=1.0, alpha=LR)

            # Pool queue: UI stationary halves, expert weights, x-hi, biases.
            t_wuh1 = wts.tile([128, 4, 512], E4, tag="wuh1")
            nc.gpsimd.dma_start(t_wuh1[:, 0:2, :], wd["wuh1"][:, 0:2, :])
            nc.gpsimd.dma_start(t_wuh1[:, 2:4, :], wd["wuh1"][:, 2:4, :])
            t_wh1 = wts.tile([128, 4, 512], E4, tag="wh1")
            nc.gpsimd.dma_start(t_wh1[:], wd["wh1"])
            for lo, hi in first:
                nc.gpsimd.dma_start(t_xh[:, :, lo:hi], xh_d[:, :, lo:hi])
            t_wl1 = wts.tile([128, 4, 512], E4, tag="wl1")
            nc.gpsimd.dma_start(t_wl1[:], wd["wl1"])
            t_iota = wts.tile([128, 4], F32, tag="iota")
            nc.gpsimd.dma_start(t_iota[:], iota_d)
            for lo, hi in rest:
                nc.gpsimd.dma_start(t_xh[:, :, lo:hi], xh_d[:, :, lo:hi])
            t_wh2 = wts.tile([128, 4, 256], E4, tag="wh2")
            nc.gpsimd.dma_start(t_wh2[:], wd["wh2"])
            t_wl2 = wts.tile([128, 4, 256], E4, tag="wl2")
            nc.gpsimd.dma_start(t_wl2[:], wd["wl2"])
            t_wh3 = wts.tile([128, 2, 128], E4, tag="wh3")
            nc.gpsimd.dma_start(t_wh3[:], wd["wh3"])
            t_wl3 = wts.tile([128, 2, 128], E4, tag="wl3")
            nc.gpsimd.dma_start(t_wl3[:], wd["wl3"])

            ones = uip.tile([128, 1], BF16, tag="ones")
            nc.vector.memset(ones[:], 1.0)
            ident = uip.tile([128, 128], F32, tag="ident")
            make_identity(nc, ident[:])

            # ---- emission helpers ----
            def mm3(ps, whT, wlT, mvh, mvl, pair, m, lo, hi):
                """3-term fp8 DoubleRow group piece for one (m, colhalf)."""
                wsl_h = whT[:, 2 * pair:2 * pair + 2, m * 128:(m + 1) * 128]
                wsl_l = wlT[:, 2 * pair:2 * pair + 2, m * 128:(m + 1) * 128]
                xs_h = mvh[:, 2 * pair:2 * pair + 2, lo:hi]
                xs_l = mvl[:, 2 * pair:2 * pair + 2, lo:hi]
                o = ps[:, lo:hi]
                nc.tensor.matmul(o, wsl_h, xs_h, start=(pair == 0),
                                 stop=False, perf_mode=DR)
                nc.tensor.matmul(o, wsl_l, xs_h, start=False, stop=False,
                                 perf_mode=DR)
                nc.tensor.matmul(o, wsl_h, xs_l, start=False,
                                 stop=(pair == 1), perf_mode=DR)

            def mm2(ps, whT, wlT, mv, pairs, m, lo, hi):
                """2-term fp8 DoubleRow group: (wh+wl)*h."""
                o = ps[:, lo:hi]
                for p in range(pairs):
                    wsl_h = whT[:, 2 * p:2 * p + 2, m * 128:(m + 1) * 128]
                    wsl_l = wlT[:, 2 * p:2 * p + 2, m * 128:(m + 1) * 128]
                    xs = mv[:, 2 * p:2 * p + 2, lo:hi]
                    nc.tensor.matmul(o, wsl_h, xs, start=(p == 0),
                                     stop=False, perf_mode=DR)
                    nc.tensor.matmul(o, wsl_l, xs, start=False,
                                     stop=(p == pairs - 1), perf_mode=DR)

            S16 = 1.0 / WSCALE

            # Activation scale propagation (all biases are zero):
            #   h1_stored = lrelu(psum1)        = 16*h1   (Act scale 1)
            #   h2_stored = lrelu(psum2/16)     = 16*h2   (Act scale 1/16)
            #   h3_stored = lrelu(psum3)        = 256*h3  (DVE 1-op stt)
            #   pred comes out 256x; host divides.
            # ---------------- expert chunk pipeline ----------------
            ps_d1 = [[None, None] for _ in range(NCH)]   # L1 m01 / m23
            ps_d2 = [None] * NCH                         # L2 m01
            ps_l3 = [None] * NCH
            ps_dot = [None] * NCH
            h1 = [None] * NCH
            h2 = [None] * NCH
            h3 = [None] * NCH
            ohs = [None] * NCH
            prods = [None] * NCH
            pcs = [None] * NCH

            def alloc_chunk(ci):
                ps_d1[ci][0] = mmd.tile([128, 2, 512], F32, tag="mmd",
                                        name=f"d1a_c{ci}")
                ps_d1[ci][1] = mmd.tile([128, 2, 512], F32, tag="mmd",
                                        name=f"d1b_c{ci}")
                ps_d2[ci] = mmd.tile([128, 2, 512], F32, tag="mmd",
                                     name=f"d2_c{ci}")
                ps_l3[ci] = mm1.tile([128, 512], F32, tag="mm",
                                     name=f"ps3c{ci}")
                ps_dot[ci] = mm1.tile([128, 512], F32, tag="mm",
                                      name=f"psdc{ci}")
                h1[ci] = actp.tile([128, 4, 512], E4, tag="h1",
                                   name=f"h1c{ci}")
                h2[ci] = actp.tile([128, 2, 512], E4, tag="h2",
                                   name=f"h2c{ci}")
                h3[ci] = actp.tile([128, 1, 512], BF16, tag="h3",
                                   name=f"h3c{ci}")
                ohs[ci] = finp.tile([128, 4, 512], BF16, tag="oh",
                                    name=f"oh{ci}")
                prods[ci] = finp.tile([128, 512], BF16, tag="prod",
                                      name=f"prod{ci}")
                pcs[ci] = finp.tile([1, 512], F32, tag="pc", name=f"pc{ci}")

            def e_l1_group(ci, m):
                s0, n = chunks[ci]
                ps = ps_d1[ci][m // 2][:, m % 2, :]
                for lo, hi in _halves(n):
                    mm3(ps, t_wh1, t_wl1,
                        t_xh[:, :, s0:s0 + n], t_xl[:, :, s0:s0 + n],
                        0, m, lo, hi)
                    mm3(ps, t_wh1, t_wl1,
                        t_xh[:, :, s0:s0 + n], t_xl[:, :, s0:s0 + n],
                        1, m, lo, hi)

            def e_l1_act(ci, half):
                # fused pair act: [128, 2, n] psum -> fp8, scale 1 (stored 16x)
                n = chunks[ci][1]
                nc.scalar.activation(
                    h1[ci][:, 2 * half:2 * half + 2, :n],
                    ps_d1[ci][half][:, :, :n],
                    mybir.ActivationFunctionType.Lrelu,
                    bias=0.0, scale=1.0, alpha=LR)

            def e_l2_pair(ci, m):
                n = chunks[ci][1]
                ps = ps_d2[ci][:, m, :]
                for lo, hi in _halves(n):
                    mm2(ps, t_wh2, t_wl2, h1[ci][:, :, :n], 2, m, lo, hi)

            def e_l2_act(ci):
                # fused pair act, scale 1/16 (stored 16x)
                n = chunks[ci][1]
                nc.scalar.activation(
                    h2[ci][:, :, :n], ps_d2[ci][:, :, :n],
                    mybir.ActivationFunctionType.Lrelu,
                    bias=0.0, scale=S16, alpha=LR)

            def e_l3(ci):
                n = chunks[ci][1]
                for lo, hi in _halves(n):
                    mm2(ps_l3[ci], t_wh3, t_wl3, h2[ci][:, :, :n], 1, 0,
                        lo, hi)

            def e_h3_act(ci):
                # h3_stored = lrelu(psum3) = 256*h3_true (Act engine; host
                # divides pred by 256)
                n = chunks[ci][1]
                nc.scalar.activation(
                    h3[ci][:, 0, :n], ps_l3[ci][:, :n],
                    mybir.ActivationFunctionType.Lrelu,
                    bias=0.0, scale=1.0, alpha=LR)

            def e_oh(ci):
                s0, n = chunks[ci]
                for j, (g, lo, hi) in enumerate(pieces[ci]["groups"]):
                    nc.vector.tensor_scalar(
                        out=ohs[ci][:, j, lo:hi],
                        in0=t_bids[:, s0 + lo:s0 + hi],
                        scalar1=t_iota[:, g:g + 1], scalar2=None,
                        op0=mybir.AluOpType.is_equal)

            def e_psg(ci):
                mms = pieces[ci]["mms"]
                for j, (slot, lo, hi) in enumerate(mms):
                    g = pieces[ci]["groups"][slot][0]
                    nc.tensor.matmul(
                        ps_dot[ci][:, lo:hi], ui3_tok[:, g, :],
                        ohs[ci][:, slot, lo:hi],
                        start=(j == 0), stop=(j == len(mms) - 1))

            def e_prod(ci):
                n = chunks[ci][1]
                nc.vector.tensor_tensor(
                    out=prods[ci][:, :n], in0=h3[ci][:, 0, :n],
                    in1=ps_dot[ci][:, :n], op=mybir.AluOpType.mult)

            def e_psd(ci):
                n = chunks[ci][1]
                nc.tensor.matmul(ps_dot[ci][0:1, :n], ones[:],
                                 prods[ci][:, :n], start=True, stop=True)

            def e_pc_dma(ci):
                # pred copy out of PSUM on the Act engine (Lrelu alpha=1 is
                # an identity, same LUT) — keeps the DVE queue clear for
                # prod; DMA on SP (second-to-last chunk via Pool so the two
                # drain DMAs overlap).
                s0, n = chunks[ci]
                if ci == NCH - 2:
                    nc.scalar.activation(pcs[ci][:, :n], ps_dot[ci][0:1, :n],
                                         mybir.ActivationFunctionType.Lrelu,
                                         bias=0.0, scale=1.0, alpha=1.0)
                    nc.gpsimd.dma_start(pred_d[:, s0:s0 + n], pcs[ci][:, :n])
                else:
                    nc.vector.tensor_copy(pcs[ci][:, :n], ps_dot[ci][0:1, :n])
                    nc.sync.dma_start(pred_d[:, s0:s0 + n], pcs[ci][:, :n])

            for ci in range(NCH):
                alloc_chunk(ci)

            # ---------------- UI phase, interleaved with ch0/ch1 L1 ----
            ps_u1 = [mmd.tile([128, 2, 512], F32, tag="mmd", name=f"ud1{h}")
                     for h in range(2)]
            ui1 = uip.tile([128, 4, B], BF16, tag="ui1")
            # phase A: wh*xh + wl*xh (uiTl arrives late; defer wh*xl).
            # One accumulation group per m (both col-halves share the bank):
            # start on the very first mm, stop on the last phase-B mm.
            for m in range(4):
                ps = ps_u1[m // 2][:, m % 2, :]
                for lo, hi in _halves(B):
                    for pair in (0, 1):
                        wh_s = t_wuh1[:, 2 * pair:2 * pair + 2,
                                      m * 128:(m + 1) * 128]
                        wl_s = t_wul1[:, 2 * pair:2 * pair + 2,
                                      m * 128:(m + 1) * 128]
                        xh_s = t_uiTh[:, 2 * pair:2 * pair + 2, lo:hi]
                        nc.tensor.matmul(ps[:, lo:hi], wh_s, xh_s,
                                         start=(pair == 0 and lo == 0),
                                         stop=False, perf_mode=DR)
                        nc.tensor.matmul(ps[:, lo:hi], wl_s, xh_s,
                                         start=False, stop=False,
                                         perf_mode=DR)
            # phase B: wh*xl, close groups, fused pair acts
            for m in range(4):
                ps = ps_u1[m // 2][:, m % 2, :]
                for lo, hi in _halves(B):
                    for pair in (0, 1):
                        wh_s = t_wuh1[:, 2 * pair:2 * pair + 2,
                                      m * 128:(m + 1) * 128]
                        xl_s = t_uiTl[:, 2 * pair:2 * pair + 2, lo:hi]
                        nc.tensor.matmul(ps[:, lo:hi], wh_s, xl_s,
                                         start=False,
                                         stop=(pair == 1 and hi == B),
                                         perf_mode=DR)
                if m % 2 == 1:
                    nc.scalar.activation(
                        ui1[:, m - 1:m + 1, :], ps_u1[m // 2][:, :, :B],
                        mybir.ActivationFunctionType.Lrelu,
                        bias=0.0, scale=S16, alpha=LR)

            # fill the ui1-act latency: ch0 L1 m0/m1
            e_l1_group(0, 0)
            e_l1_group(0, 1)

            ps_u2 = mmd.tile([128, 2, 512], F32, tag="mmd", name="ud2")
            ui2 = uip.tile([128, 2, B], BF16, tag="ui2")
            for k in range(4):
                for m in range(2):
                    nc.tensor.matmul(
                        ps_u2[:, m, :B], t_wu2[:, k, m * 128:(m + 1) * 128],
                        ui1[:, k, :], start=(k == 0), stop=(k == 3))
            nc.scalar.activation(ui2[:, :, :], ps_u2[:, :, :B],
                                 mybir.ActivationFunctionType.Lrelu,
                                 bias=0.0, scale=1.0, alpha=LR)

            # fillers while ui2 act drains: rest of ch0 L1 + ch1 L1 start
            e_l1_act(0, 0)
            e_l1_group(0, 2)
            e_l1_group(0, 3)
            e_l1_act(0, 1)
            e_l1_group(1, 0)

            ps_u3 = mm1.tile([128, 512], F32, tag="mm", name="psu3")
            ui3 = uip.tile([128, 1, B], F32, tag="ui3")
            for k in range(2):
                nc.tensor.matmul(ps_u3[:, :B], t_wu3[:, k, :], ui2[:, k, :],
                                 start=(k == 0), stop=(k == 1))
            nc.scalar.activation(ui3[:, 0, :], ps_u3[:, :B],
                                 mybir.ActivationFunctionType.Lrelu,
                                 bias=0.0, scale=1.0, alpha=LR)

            e_l1_group(1, 1)
            e_l1_act(1, 0)
            e_l1_group(1, 2)

            # transpose ui3 -> token-major [128b, 4, 128d]
            tps = mm1.tile([128, 512], F32, tag="mm", name="tps")
            for c in range(4):
                nc.tensor.transpose(tps[:, c * 128:(c + 1) * 128],
                                    ui3[:, 0, c * 128:(c + 1) * 128], ident[:])
            ui3_tok = uip.tile([128, 4, 128], BF16, tag="ui3tok")
            nc.vector.tensor_copy(ui3_tok[:], tps[:])
            for ci in range(NCH):
                e_oh(ci)

            # ch0 through L2/L3; finish ch1 L1
            e_l2_pair(0, 0)
            e_l2_pair(0, 1)
            e_l2_act(0)
            e_l1_group(1, 3)
            e_l1_act(1, 1)
            e_l3(0)
            e_h3_act(0)

            # ---- steady periods ----
            # period p: L1(p+1) | L2+L3(p) | psg/prod/psd/pc(p-1)
            for p in range(1, NCH - 1):
                cl1 = p + 1 if p + 1 < NCH else None
                cl2 = p
                cfin = p - 1
                if cl1 is not None:
                    e_l1_group(cl1, 0)
                e_l2_pair(cl2, 0)
                e_psg(cfin)
                if cl1 is not None:
                    e_l1_group(cl1, 1)
                e_l2_pair(cl2, 1)
                e_l2_act(cl2)
                if cl1 is not None:
                    e_l1_act(cl1, 0)
                e_prod(cfin)
                if cl1 is not None:
                    e_l1_group(cl1, 2)
                e_psd(cfin)
                e_l3(cl2)
                e_h3_act(cl2)
                if cl1 is not None:
                    e_l1_group(cl1, 3)
                    e_l1_act(cl1, 1)
                e_pc_dma(cfin)

            # ---- fused drain: last two chunks' chains interleaved ----
            ca, cb = NCH - 2, NCH - 1
            e_l2_pair(cb, 0)
            e_l2_pair(cb, 1)
            e_l2_act(cb)
            e_psg(ca)
            e_prod(ca)          # DVE: h3(ca) act emitted in the last period
            e_l3(cb)
            e_h3_act(cb)        # Act
            e_psg(cb)
            e_psd(ca)
            e_prod(cb)          # DVE
            e_pc_dma(ca)
            e_psd(cb)
            e_pc_dma(cb)

    nc.compile()
    return nc


def _prepare(u_emb, i_emb, a_emb, o_emb, s):
    """Host-side sharding + fp8 hi/lo splits + layouts."""
    s_flat = np.asarray(s).reshape(-1).astype(np.int64)
    n_tokens = s_flat.shape[0]
    X = np.concatenate(
        [np.asarray(a_emb, dtype=np.float32).reshape(n_tokens, IN1),
         np.asarray(o_emb, dtype=np.float32).reshape(n_tokens, IN1)],
        axis=1)
    uiT = np.ascontiguousarray(
        np.concatenate([np.asarray(u_emb, dtype=np.float32),
                        np.asarray(i_emb, dtype=np.float32)], axis=1).T)

    idx = [np.flatnonzero(s_flat == e) for e in range(R)]
    tp = max(520, -(-max(max(len(ix) for ix in idx), 1) // 8) * 8)
    chunks = _chunks(tp)

    iota4 = np.ascontiguousarray(
        (np.arange(128, dtype=np.float32)[:, None]
         + 128.0 * np.arange(4, dtype=np.float32)[None, :]))

    def lay(a, kc):
        return np.ascontiguousarray(a.reshape(kc, 128, -1).transpose(1, 0, 2))

    uiTh = uiT.astype(E4_NP)
    uiTl = (uiT - uiTh.astype(np.float32)).astype(E4_NP)

    in_maps = []
    runs_all = [[] for _ in chunks]
    for e in range(R):
        order = np.argsort(idx[e] // N_TOK, kind="stable")
        idx[e] = idx[e][order]
        ix = idx[e]
        pad = np.full(tp, n_tokens - 1, dtype=np.int64)
        pad[:len(ix)] = ix
        xT = np.ascontiguousarray(X[pad].T)          # [512, tp]
        xh = xT.astype(E4_NP)
        xl = (xT - xh.astype(np.float32)).astype(E4_NP)
        b_of_tok = pad // N_TOK
        gid = b_of_tok // 128
        bids = np.ascontiguousarray(np.broadcast_to(
            b_of_tok.astype(np.float16)[None, :], (128, tp)))
        for ci, (s0, n) in enumerate(chunks):
            g = gid[s0:s0 + n]
            start = 0
            for j in range(1, n + 1):
                if j == n or g[j] != g[start]:
                    runs_all[ci].append((int(g[start]), start, j))
                    start = j
        in_maps.append({"xh": lay(xh, 4), "xl": lay(xl, 4), "bids": bids,
                        "iota4": iota4, "uiTh": lay(uiTh, 4),
                        "uiTl": lay(uiTl, 4)})

    # union spans per (chunk, group) across cores; psg matmuls split at
    # overlap boundaries (each mm range uniformly fresh or accumulating)
    pieces = []
    for ci in range(len(chunks)):
        spans = {}
        for g, lo, hi in runs_all[ci]:
            if g in spans:
                spans[g] = (min(spans[g][0], lo), max(spans[g][1], hi))
            else:
                spans[g] = (lo, hi)
        groups = sorted((g, lo, hi) for g, (lo, hi) in spans.items())
        bounds = sorted({b for _, lo, hi in groups for b in (lo, hi)})
        mms = []
        for a, b in zip(bounds[:-1], bounds[1:]):
            for slot, (g, lo, hi) in enumerate(groups):
                if lo <= a and b <= hi:
                    mms.append((slot, a, b))
        pieces.append({"groups": groups, "mms": mms})

    return in_maps, idx, tp, pieces


def _split16(w):
    ws = np.asarray(w, dtype=np.float32) * WSCALE
    wh = ws.astype(E4_NP)
    wl = (ws - wh.astype(np.float32)).astype(E4_NP)
    return wh, wl


def _lay(a, kc):
    return np.ascontiguousarray(
        np.asarray(a).reshape(kc, 128, -1).transpose(1, 0, 2))


def _add_weights(m, e, W1, W2, W3, Wu1, Wu2, Wu3, wu_cache):
    for nm, w, kc in (("w1", W1[e], 4), ("w2", W2[e], 4), ("w3", W3[e], 2)):
        wh, wl = _split16(w)
        m[nm.replace("w", "wh")] = _lay(wh, kc)
        m[nm.replace("w", "wl")] = _lay(wl, kc)
    if not wu_cache:
        wh, wl = _split16(Wu1)
        wu_cache["wuh1"] = _lay(wh, 4)
        wu_cache["wul1"] = _lay(wl, 4)
        wu_cache["wu2"] = _lay(
            np.asarray(Wu2, dtype=np.float32).astype(BF16_NP), 4)
        wu_cache["wu3"] = _lay(
            np.asarray(Wu3, dtype=np.float32).astype(BF16_NP), 2)
    m.update(wu_cache)


def kernel(u_emb, i_emb, a_emb, o_emb, s,
           W1, b1, W2, b2, W3, b3,
           Wu1, bu1, Wu2, bu2, Wu3, bu3):
    for b in (b1, b2, b3, bu1, bu2, bu3):
        assert not np.any(np.asarray(b)), "kernel assumes zero biases"
    in_maps, idx, tp, pieces = _prepare(u_emb, i_emb, a_emb, o_emb, s)
    wu_cache = {}
    for e in range(R):
        _add_weights(in_maps[e], e, W1, W2, W3, Wu1, Wu2, Wu3, wu_cache)

    nc = _build_kernel(tp, pieces)
    res = run_bass_kernel_spmd(nc, in_maps, core_ids=list(range(N_CORES)))

    s_arr = np.asarray(s)
    out = np.zeros(s_arr.size, dtype=np.float32)
    for e in range(R):
        pred = res.results[e]["pred"].reshape(-1)
        out[idx[e]] = pred[:len(idx[e])] * (1.0 / 256.0)
    return out.reshape(s_arr.shape)


# revision 43
# speedup vs baseline: 1.0032x; 1.0032x over previous
"""AOSPredictionLayer — Trainium2 Bass kernel (8 NeuronCores, expert-sharded).

pred[b,n] = <ui_mlp(concat(u,i))[b], relation_mlp[s[b,n]](concat(a,o)[b,n])>
B=512, N=32, R=8, MLP dims 512->512->256->128 (leaky_relu 0.01, zero biases).

MoE routing: core e gets the tokens of relation e (sorted by batch row,
padded to tp); per-relation weights sharded, UI MLP replicated.

Numerics (all biases are zero; scales fold into activations / host):
  - L1 (expert + UI) as fp8e4m3 DoubleRow (0.5 cyc/col, 2x128 contraction
    per instr), 3-term error-compensated: w' = 16*W split hi/lo, x split
    hi/lo on host; wh*xh + wl*xh + wh*xl accumulate in fp32 PSUM.  Recovers
    bf16-level accuracy at 0.75x the bf16 matmul cost.
  - Expert L2/L3 as fp8 DoubleRow 2-term (wh+wl)*h, h1/h2 written directly
    in fp8 by the activations; UI L2/L3 stay bf16 for accuracy headroom.
  - Activation scale chain: h1_stored=lrelu(16*W1x) [scale 1],
    h2_stored=lrelu(psum/16) [1/16], h3_stored=lrelu(psum)=256*h3;
    host divides pred by 256.  Measured rel err ~1.38e-2 on HW.

Dataflow per core, in 512-token chunks (order: 256a, fulls..., 256b, tail):
  - one-hot rows oh_c = (b == iota+128c) on DVE; psg = ui3_tok^T @ oh
    accumulated across the chunk's b-groups in ONE PSUM group (overlapping
    union spans across cores add; wrong-group terms are zero);
    prod = h3 * psg on DVE; pred = ones^T @ prod; copy + DMA out.
  - Software pipeline, period p emits PE work for three chunks at once:
    L1(p+1) | L2+L3(p) | psg/prod/psd/pc(p-1), so the in-order PE stream
    never waits; the last two chunks drain with interleaved chains.
  - PSUM: L1 pairs in [128,2,512] double-bank tiles (fused pair
    activations), L2 pair likewise; acts on the Act engine, h3 too; DVE
    does one-hots/prod/pc; weights+x arrive split across SP/Act/Pool DMA
    queues with UI-critical halves first.
"""
import sys

for _p in ("/opt/trn_rl_repo", "/opt/pypackages"):
    if _p not in sys.path:
        sys.path.append(_p)

import numpy as np
import ml_dtypes

import concourse.tile as tile
from concourse import bacc, mybir
from concourse.masks import make_identity
from concourse.bass_utils import run_bass_kernel_spmd

F32 = mybir.dt.float32
BF16 = mybir.dt.bfloat16
F16 = mybir.dt.float16
E4 = mybir.dt.float8e4

BF16_NP = ml_dtypes.bfloat16
E4_NP = ml_dtypes.float8_e4m3

B = 512
N_TOK = 32
IN1 = 256
HID = [512, 256, 128]
R = 8
N_CORES = 8

D_AO = 2 * IN1                     # 512
DIMS = [D_AO] + HID                # [512, 512, 256, 128]
WSCALE = 16.0
DR = mybir.MatmulPerfMode.DoubleRow
LR = 0.01                          # leaky-relu slope
BOFF = [0, 4, 6]                   # bias col offset per layer in [128,7]


def _chunks(tp):
    """Processing order: first full chunk split half (256a), then fulls,
    then 256b, tiny tail LAST (shortest drain chain)."""
    fulls = []
    s = 0
    while tp - s > 512:
        fulls.append((s, 512))
        s += 512
    tail = (s, tp - s)
    la, lb = fulls[-1]
    return [(la, 256)] + fulls[:-1] + [(la + 256, 256), tail]


def _halves(n):
    if n <= 256:
        return [(0, n)]
    return [(0, 256), (256, n)]


def _build_kernel(tp, pieces):
    """pieces[ci] = list of (group, lo, hi): union spans of b//128 groups
    in chunk ci across all cores (one-hot zeroes wrong-group terms)."""
    nc = bacc.Bacc("TRN2", target_bir_lowering=False, debug=False,
                   num_devices=N_CORES)

    xh_d = nc.dram_tensor("xh", [128, 4, tp], E4, kind="ExternalInput").ap()
    xl_d = nc.dram_tensor("xl", [128, 4, tp], E4, kind="ExternalInput").ap()
    wd = {}
    for nm, shp in (("wh1", [128, 4, 512]), ("wl1", [128, 4, 512]),
                    ("wh2", [128, 4, 256]), ("wl2", [128, 4, 256]),
                    ("wh3", [128, 2, 128]), ("wl3", [128, 2, 128]),
                    ("uiTh", [128, 4, B]), ("uiTl", [128, 4, B]),
                    ("wuh1", [128, 4, 512]), ("wul1", [128, 4, 512])):
        wd[nm] = nc.dram_tensor(nm, shp, E4, kind="ExternalInput").ap()
    wu2_d = nc.dram_tensor("wu2", [128, 4, 256], BF16,
                           kind="ExternalInput").ap()
    wu3_d = nc.dram_tensor("wu3", [128, 2, 128], BF16,
                           kind="ExternalInput").ap()
    bids_d = nc.dram_tensor("bids", [128, tp], F16, kind="ExternalInput").ap()
    iota_d = nc.dram_tensor("iota4", [128, 4], F32, kind="ExternalInput").ap()
    pred_d = nc.dram_tensor("pred", [1, tp], F32, kind="ExternalOutput").ap()

    chunks = _chunks(tp)
    NCH = len(chunks)

    with tile.TileContext(nc) as tc:
        with (
            tc.tile_pool(name="wts", bufs=1) as wts,
            tc.tile_pool(name="xin", bufs=1) as xin,
            tc.tile_pool(name="uip", bufs=1) as uip,
            tc.tile_pool(name="actp", bufs=2) as actp,
            tc.tile_pool(name="finp", bufs=2) as finp,
            tc.tile_pool(name="mmd", bufs=3, space="PSUM") as mmd,
            tc.tile_pool(name="mm1", bufs=2, space="PSUM") as mm1,
        ):
            # ---- tiny PE warm-up (pins pe_busy_start; ~free) ----
            dum = uip.tile([1, 8], BF16, tag="dum")
            nc.vector.memset(dum[:], 0.0)
            psw = mm1.tile([128, 512], F32, tag="mm", name="psw")
            for _ in range(2):
                nc.tensor.matmul(psw[0:1, 0:8], dum[0:1, 0:1], dum[:],
                                 start=True, stop=True)

            # ---- DMAs ----
            # SP queue: UI-phase-critical tensors in half (k-pair) pieces so
            # the first UI matmuls can start ~1.9us in.
            t_uiTh = wts.tile([128, 4, B], E4, tag="uiTh")
            nc.sync.dma_start(t_uiTh[:, 0:2, :], wd["uiTh"][:, 0:2, :])
            t_wul1 = wts.tile([128, 4, 512], E4, tag="wul1")
            nc.sync.dma_start(t_wul1[:, 0:2, :], wd["wul1"][:, 0:2, :])
            nc.sync.dma_start(t_uiTh[:, 2:4, :], wd["uiTh"][:, 2:4, :])
            nc.sync.dma_start(t_wul1[:, 2:4, :], wd["wul1"][:, 2:4, :])
            t_uiTl = wts.tile([128, 4, B], E4, tag="uiTl")
            nc.sync.dma_start(t_uiTl[:], wd["uiTl"])
            t_xl = xin.tile([128, 4, tp], E4, tag="xl")
            t_xh = xin.tile([128, 4, tp], E4, tag="xh")
            # pre-loop chunks (chunks[0], chunks[1]) first, rest later
            c0s, c0n = None, None

            def x_slices():
                (a0, n0), (a1, n1) = chunks[0], chunks[1]
                lo = min(a0, a1)
                hi = max(a0 + n0, a1 + n1)
                if a0 + n0 == a1 or a1 + n1 == a0:
                    return ([(lo, hi)],
                            [(r, s) for r, s in ((0, lo), (hi, tp)) if s > r])
                return ([(a0, a0 + n0), (a1, a1 + n1)],
                        [(r, s) for r, s in ((0, min(a0, a1)),
                                             (min(a0 + n0, a1 + n1),
                                              max(a0, a1)),
                                             (max(a0 + n0, a1 + n1), tp))
                         if s > r])

            first, rest = x_slices()
            for lo, hi in first:
                nc.sync.dma_start(t_xl[:, :, lo:hi], xl_d[:, :, lo:hi])
            for lo, hi in rest:
                nc.sync.dma_start(t_xl[:, :, lo:hi], xl_d[:, :, lo:hi])
            t_bids = wts.tile([128, tp], F16, tag="bids")
            nc.sync.dma_start(t_bids[:], bids_d)

            # Act queue: UI L2/L3 weights + LUT warm-up (table load gets
            # hoisted to the queue head).
            t_wu2 = wts.tile([128, 4, 256], BF16, tag="wu2")
            nc.scalar.dma_start(t_wu2[:], wu2_d)
            t_wu3 = wts.tile([128, 2, 128], BF16, tag="wu3")
            nc.scalar.dma_start(t_wu3[:], wu3_d)
            lut = uip.tile([1, 1], F32, tag="lut")
            nc.vector.memset(lut[:], 0.0)
            nc.scalar.activation(lut[:], lut[:],
                                 mybir.ActivationFunctionType.Lrelu,
                                 bias=0.0, scale=1.0, alpha=LR)

            # Pool queue: UI stationary halves, expert weights, x-hi, biases.
            t_wuh1 = wts.tile([128, 4, 512], E4, tag="wuh1")
            nc.gpsimd.dma_start(t_wuh1[:, 0:2, :], wd["wuh1"][:, 0:2, :])
            nc.gpsimd.dma_start(t_wuh1[:, 2:4, :], wd["wuh1"][:, 2:4, :])
            t_wh1 = wts.tile([128, 4, 512], E4, tag="wh1")
            nc.gpsimd.dma_start(t_wh1[:], wd["wh1"])
            for lo, hi in first:
                nc.gpsimd.dma_start(t_xh[:, :, lo:hi], xh_d[:, :, lo:hi])
            t_wl1 = wts.tile([128, 4, 512], E4, tag="wl1")
            nc.gpsimd.dma_start(t_wl1[:], wd["wl1"])
            t_iota = wts.tile([128, 4], F32, tag="iota")
            nc.gpsimd.dma_start(t_iota[:], iota_d)
            for lo, hi in rest:
                nc.gpsimd.dma_start(t_xh[:, :, lo:hi], xh_d[:, :, lo:hi])
            t_wh2 = wts.tile([128, 4, 256], E4, tag="wh2")
            nc.gpsimd.dma_start(t_wh2[:], wd["wh2"])
            t_wl2 = wts.tile([128, 4, 256], E4, tag="wl2")
            nc.gpsimd.dma_start(t_wl2[:], wd["wl2"])
            t_wh3 = wts.tile([128, 2, 128], E4, tag="wh3")
            nc.gpsimd.dma_start(t_wh3[:], wd["wh3"])
            t_wl3 = wts.tile([128, 2, 128], E4, tag="wl3")
            nc.gpsimd.dma_start(t_wl3[:], wd["wl3"])

            ones = uip.tile([128, 1], BF16, tag="ones")
            nc.vector.memset(ones[:], 1.0)
            ident = uip.tile([128, 128], F32, tag="ident")
            make_identity(nc, ident[:])

            # ---- emission helpers ----
            def mm3(ps, whT, wlT, mvh, mvl, pair, m, lo, hi):
                """3-term fp8 DoubleRow group piece for one (m, colhalf)."""
                wsl_h = whT[:, 2 * pair:2 * pair + 2, m * 128:(m + 1) * 128]
                wsl_l = wlT[:, 2 * pair:2 * pair + 2, m * 128:(m + 1) * 128]
                xs_h = mvh[:, 2 * pair:2 * pair + 2, lo:hi]
                xs_l = mvl[:, 2 * pair:2 * pair + 2, lo:hi]
                o = ps[:, lo:hi]
                nc.tensor.matmul(o, wsl_h, xs_h, start=(pair == 0),
                                 stop=False, perf_mode=DR)
                nc.tensor.matmul(o, wsl_l, xs_h, start=False, stop=False,
                                 perf_mode=DR)
                nc.tensor.matmul(o, wsl_h, xs_l, start=False,
                                 stop=(pair == 1), perf_mode=DR)

            def mm2(ps, whT, wlT, mv, pairs, m, lo, hi):
                """2-term fp8 DoubleRow group: (wh+wl)*h."""
                o = ps[:, lo:hi]
                for p in range(pairs):
                    wsl_h = whT[:, 2 * p:2 * p + 2, m * 128:(m + 1) * 128]
                    wsl_l = wlT[:, 2 * p:2 * p + 2, m * 128:(m + 1) * 128]
                    xs = mv[:, 2 * p:2 * p + 2, lo:hi]
                    nc.tensor.matmul(o, wsl_h, xs, start=(p == 0),
                                     stop=False, perf_mode=DR)
                    nc.tensor.matmul(o, wsl_l, xs, start=False,
                                     stop=(p == pairs - 1), perf_mode=DR)

            S16 = 1.0 / WSCALE

            # Activation scale propagation (all biases are zero):
            #   h1_stored = lrelu(psum1)        = 16*h1   (Act scale 1)
            #   h2_stored = lrelu(psum2/16)     = 16*h2   (Act scale 1/16)
            #   h3_stored = lrelu(psum3)        = 256*h3  (DVE 1-op stt)
            #   pred comes out 256x; host divides.
            # ---------------- expert chunk pipeline ----------------
            ps_d1 = [[None, None] for _ in range(NCH)]   # L1 m01 / m23
            ps_d2 = [None] * NCH                         # L2 m01
            ps_l3 = [None] * NCH
            ps_dot = [None] * NCH
            h1 = [None] * NCH
            h2 = [None] * NCH
            h3 = [None] * NCH
            ohs = [None] * NCH
            prods = [None] * NCH
            pcs = [None] * NCH

            def alloc_chunk(ci):
                ps_d1[ci][0] = mmd.tile([128, 2, 512], F32, tag="mmd",
                                        name=f"d1a_c{ci}")
                ps_d1[ci][1] = mmd.tile([128, 2, 512], F32, tag="mmd",
                                        name=f"d1b_c{ci}")
                ps_d2[ci] = mmd.tile([128, 2, 512], F32, tag="mmd",
                                     name=f"d2_c{ci}")
                ps_l3[ci] = mm1.tile([128, 512], F32, tag="mm",
                                     name=f"ps3c{ci}")
                ps_dot[ci] = mm1.tile([128, 512], F32, tag="mm",
                                      name=f"psdc{ci}")
                h1[ci] = actp.tile([128, 4, 512], E4, tag="h1",
                                   name=f"h1c{ci}")
                h2[ci] = actp.tile([128, 2, 512], E4, tag="h2",
                                   name=f"h2c{ci}")
                h3[ci] = actp.tile([128, 1, 512], BF16, tag="h3",
                                   name=f"h3c{ci}")
                ohs[ci] = finp.tile([128, 4, 512], BF16, tag="oh",
                                    name=f"oh{ci}")
                prods[ci] = finp.tile([128, 512], BF16, tag="prod",
                                      name=f"prod{ci}")
                pcs[ci] = finp.tile([1, 512], F32, tag="pc", name=f"pc{ci}")

            def e_l1_group(ci, m):
                s0, n = chunks[ci]
                ps = ps_d1[ci][m // 2][:, m % 2, :]
                for lo, hi in _halves(n):
                    mm3(ps, t_wh1, t_wl1,
                        t_xh[:, :, s0:s0 + n], t_xl[:, :, s0:s0 + n],
                        0, m, lo, hi)
                    mm3(ps, t_wh1, t_wl1,
                        t_xh[:, :, s0:s0 + n], t_xl[:, :, s0:s0 + n],
                        1, m, lo, hi)

            def e_l1_act(ci, half):
                # fused pair act: [128, 2, n] psum -> fp8, scale 1 (stored 16x)
                n = chunks[ci][1]
                nc.scalar.activation(
                    h1[ci][:, 2 * half:2 * half + 2, :n],
                    ps_d1[ci][half][:, :, :n],
                    mybir.ActivationFunctionType.Lrelu,
                    bias=0.0, scale=1.0, alpha=LR)

            def e_l2_pair(ci, m):
                n = chunks[ci][1]
                ps = ps_d2[ci][:, m, :]
                for lo, hi in _halves(n):
                    mm2(ps, t_wh2, t_wl2, h1[ci][:, :, :n], 2, m, lo, hi)

            def e_l2_act(ci):
                # fused pair act, scale 1/16 (stored 16x)
                n = chunks[ci][1]
                nc.scalar.activation(
                    h2[ci][:, :, :n], ps_d2[ci][:, :, :n],
                    mybir.ActivationFunctionType.Lrelu,
                    bias=0.0, scale=S16, alpha=LR)

            def e_l3(ci):
                n = chunks[ci][1]
                for lo, hi in _halves(n):
                    mm2(ps_l3[ci], t_wh3, t_wl3, h2[ci][:, :, :n], 1, 0,
                        lo, hi)

            def e_h3_act(ci):
                # h3_stored = lrelu(psum3) = 256*h3_true (Act engine; host
                # divides pred by 256)
                n = chunks[ci][1]
                nc.scalar.activation(
                    h3[ci][:, 0, :n], ps_l3[ci][:, :n],
                    mybir.ActivationFunctionType.Lrelu,
                    bias=0.0, scale=1.0, alpha=LR)

            def e_oh(ci):
                s0, n = chunks[ci]
                for j, (g, lo, hi) in enumerate(pieces[ci]["groups"]):
                    nc.vector.tensor_scalar(
                        out=ohs[ci][:, j, lo:hi],
                        in0=t_bids[:, s0 + lo:s0 + hi],
                        scalar1=t_iota[:, g:g + 1], scalar2=None,
                        op0=mybir.AluOpType.is_equal)

            def e_psg(ci):
                mms = pieces[ci]["mms"]
                for j, (slot, lo, hi) in enumerate(mms):
                    g = pieces[ci]["groups"][slot][0]
                    nc.tensor.matmul(
                        ps_dot[ci][:, lo:hi], ui3_tok[:, g, :],
                        ohs[ci][:, slot, lo:hi],
                        start=(j == 0), stop=(j == len(mms) - 1))

            def e_prod(ci):
                n = chunks[ci][1]
                nc.vector.tensor_tensor(
                    out=prods[ci][:, :n], in0=h3[ci][:, 0, :n],
                    in1=ps_dot[ci][:, :n], op=mybir.AluOpType.mult)

            def e_psd(ci):
                n = chunks[ci][1]
                nc.tensor.matmul(ps_dot[ci][0:1, :n], ones[:],
                                 prods[ci][:, :n], start=True, stop=True)

            def e_pc_dma(ci):
                # pred copy out of PSUM on the Act engine (Lrelu alpha=1 is
                # an identity, same LUT) — keeps the DVE queue clear for
                # prod; DMA on SP (second-to-last chunk via Pool so the two
                # drain DMAs overlap).
                s0, n = chunks[ci]
                if ci == NCH - 2:
                    nc.scalar.activation(pcs[ci][:, :n], ps_dot[ci][0:1, :n],
                                         mybir.ActivationFunctionType.Lrelu,
                                         bias=0.0, scale=1.0, alpha=1.0)
                    nc.gpsimd.dma_start(pred_d[:, s0:s0 + n], pcs[ci][:, :n])
                else:
                    nc.vector.tensor_copy(pcs[ci][:, :n], ps_dot[ci][0:1, :n])
                    nc.sync.dma_start(pred_d[:, s0:s0 + n], pcs[ci][:, :n])

            for ci in range(NCH):
                alloc_chunk(ci)

            # ---------------- UI phase, interleaved with ch0/ch1 L1 ----
            ps_u1 = [mmd.tile([128, 2, 512], F32, tag="mmd", name=f"ud1{h}")
                     for h in range(2)]
            ui1 = uip.tile([128, 4, B], BF16, tag="ui1")
            # phase A: wh*xh + wl*xh (uiTl arrives late; defer wh*xl).
            # One accumulation group per m (both col-halves share the bank):
            # start on the very first mm, stop on the last phase-B mm.
            for m in range(4):
                ps = ps_u1[m // 2][:, m % 2, :]
                for lo, hi in _halves(B):
                    for pair in (0, 1):
                        wh_s = t_wuh1[:, 2 * pair:2 * pair + 2,
                                      m * 128:(m + 1) * 128]
                        wl_s = t_wul1[:, 2 * pair:2 * pair + 2,
                                      m * 128:(m + 1) * 128]
                        xh_s = t_uiTh[:, 2 * pair:2 * pair + 2, lo:hi]
                        nc.tensor.matmul(ps[:, lo:hi], wh_s, xh_s,
                                         start=(pair == 0 and lo == 0),
                                         stop=False, perf_mode=DR)
                        nc.tensor.matmul(ps[:, lo:hi], wl_s, xh_s,
                                         start=False, stop=False,
                                         perf_mode=DR)
            # phase B: wh*xl, close groups, fused pair acts
            for m in range(4):
                ps = ps_u1[m // 2][:, m % 2, :]
                for lo, hi in _halves(B):
                    for pair in (0, 1):
                        wh_s = t_wuh1[:, 2 * pair:2 * pair + 2,
                                      m * 128:(m + 1) * 128]
                        xl_s = t_uiTl[:, 2 * pair:2 * pair + 2, lo:hi]
                        nc.tensor.matmul(ps[:, lo:hi], wh_s, xl_s,
                                         start=False,
                                         stop=(pair == 1 and hi == B),
                                         perf_mode=DR)
                if m % 2 == 1:
                    nc.scalar.activation(
                        ui1[:, m - 1:m + 1, :], ps_u1[m // 2][:, :, :B],
                        mybir.ActivationFunctionType.Lrelu,
                        bias=0.0, scale=S16, alpha=LR)

            # fill the ui1-act latency: ch0 L1 m0/m1
            e_l1_group(0, 0)
            e_l1_group(0, 1)

            ps_u2 = mmd.tile([128, 2, 512], F32, tag="mmd", name="ud2")
            ui2 = uip.tile([128, 2, B], BF16, tag="ui2")
            for k in range(4):
                for m in range(2):
                    nc.tensor.matmul(
                        ps_u2[:, m, :B], t_wu2[:, k, m * 128:(m + 1) * 128],
                        ui1[:, k, :], start=(k == 0), stop=(k == 3))
            nc.scalar.activation(ui2[:, :, :], ps_u2[:, :, :B],
                                 mybir.ActivationFunctionType.Lrelu,
                                 bias=0.0, scale=1.0, alpha=LR)

            # fillers while ui2 act drains: rest of ch0 L1 + ch1 L1 start
            e_l1_act(0, 0)
            e_l1_group(0, 2)
            e_l1_group(0, 3)
            e_l1_act(0, 1)
            e_l1_group(1, 0)

            ps_u3 = mm1.tile([128, 512], F32, tag="mm", name="psu3")
            ui3 = uip.tile([128, 1, B], F32, tag="ui3")
            for k in range(2):
                nc.tensor.matmul(ps_u3[:, :B], t_wu3[:, k, :], ui2[:, k, :],
                                 start=(k == 0), stop=(k == 1))
            nc.scalar.activation(ui3[:, 0, :], ps_u3[:, :B],
                                 mybir.ActivationFunctionType.Lrelu,
                                 bias=0.0, scale=1.0, alpha=LR)

            e_l1_group(1, 1)
            e_l1_act(1, 0)
            e_l1_group(1, 2)

            # transpose ui3 -> token-major [128b, 4, 128d]
            tps = mm1.tile([128, 512], F32, tag="mm", name="tps")
            for c in range(4):
                nc.tensor.transpose(tps[:, c * 128:(c + 1) * 128],
                                    ui3[:, 0, c * 128:(c + 1) * 128], ident[:])
            ui3_tok = uip.tile([128, 4, 128], BF16, tag="ui3tok")
            nc.vector.tensor_copy(ui3_tok[:], tps[:])
            for ci in range(NCH):
                e_oh(ci)

            # ch0 through L2/L3; finish ch1 L1
            e_l2_pair(0, 0)
            e_l2_pair(0, 1)
            e_l2_act(0)
            e_l1_group(1, 3)
            e_l1_act(1, 1)
            e_l3(0)
            e_h3_act(0)

            # ---- steady periods ----
            # period p: L1(p+1) | L2+L3(p) | psg/prod/psd/pc(p-1)
            for p in range(1, NCH - 1):
                cl1 = p + 1 if p + 1 < NCH else None
                cl2 = p
                cfin = p - 1
                if cl1 is not None:
                    e_l1_group(cl1, 0)
                e_l2_pair(cl2, 0)
                e_psg(cfin)
                if cl1 is not None:
                    e_l1_group(cl1, 1)
                e_l2_pair(cl2, 1)
                e_l2_act(cl2)
                if cl1 is not None:
                    e_l1_act(cl1, 0)
                e_prod(cfin)
                if cl1 is not None:
                    e_l1_group(cl1, 2)
                e_psd(cfin)
                e_l3(cl2)
                e_h3_act(cl2)
                if cl1 is not None:
                    e_l1_group(cl1, 3)
                    e_l1_act(cl1, 1)
                e_pc_dma(cfin)

            # ---- fused drain: last two chunks' chains interleaved ----
            ca, cb = NCH - 2, NCH - 1
            e_l2_pair(cb, 0)
            e_l2_pair(cb, 1)
            e_l2_act(cb)
            e_psg(ca)
            e_prod(ca)          # DVE: h3(ca) act emitted in the last period
            e_l3(cb)
            e_h3_act(cb)        # Act
            e_psg(cb)
            e_psd(ca)
            e_prod(cb)          # DVE
            e_pc_dma(ca)
            e_psd(cb)
            e_pc_dma(cb)

    nc.compile()
    return nc


def _prepare(u_emb, i_emb, a_emb, o_emb, s):
    """Host-side sharding + fp8 hi/lo splits + layouts."""
    s_flat = np.asarray(s).reshape(-1).astype(np.int64)
    n_tokens = s_flat.shape[0]
    X = np.concatenate(
        [np.asarray(a_emb, dtype=np.float32).reshape(n_tokens, IN1),
         np.asarray(o_emb, dtype=np.float32).reshape(n_tokens, IN1)],
        axis=1)
    uiT = np.ascontiguousarray(
        np.concatenate([np.asarray(u_emb, dtype=np.float32),
                        np.asarray(i_emb, dtype=np.float32)], axis=1).T)

    idx = [np.flatnonzero(s_flat == e) for e in range(R)]
    tp = max(520, -(-max(max(len(ix) for ix in idx), 1) // 8) * 8)
    chunks = _chunks(tp)

    iota4 = np.ascontiguousarray(
        (np.arange(128, dtype=np.float32)[:, None]
         + 128.0 * np.arange(4, dtype=np.float32)[None, :]))

    def lay(a, kc):
        return np.ascontiguousarray(a.reshape(kc, 128, -1).transpose(1, 0, 2))

    uiTh = uiT.astype(E4_NP)
    uiTl = (uiT - uiTh.astype(np.float32)).astype(E4_NP)

    in_maps = []
    runs_all = [[] for _ in chunks]
    for e in range(R):
        order = np.argsort(idx[e] // N_TOK, kind="stable")
        idx[e] = idx[e][order]
        ix = idx[e]
        pad = np.full(tp, n_tokens - 1, dtype=np.int64)
        pad[:len(ix)] = ix
        xT = np.ascontiguousarray(X[pad].T)          # [512, tp]
        xh = xT.astype(E4_NP)
        xl = (xT - xh.astype(np.float32)).astype(E4_NP)
        b_of_tok = pad // N_TOK
        gid = b_of_tok // 128
        bids = np.ascontiguousarray(np.broadcast_to(
            b_of_tok.astype(np.float16)[None, :], (128, tp)))
        for ci, (s0, n) in enumerate(chunks):
            g = gid[s0:s0 + n]
            start = 0
            for j in range(1, n + 1):
                if j == n or g[j] != g[start]:
                    runs_all[ci].append((int(g[start]), start, j))
                    start = j
        in_maps.append({"xh": lay(xh, 4), "xl": lay(xl, 4), "bids": bids,
                        "iota4": iota4, "uiTh": lay(uiTh, 4),
                        "uiTl": lay(uiTl, 4)})

    # union spans per (chunk, group) across cores; psg matmuls split at
    # overlap boundaries (each mm range uniformly fresh or accumulating)
    pieces = []
    for ci in range(len(chunks)):
        spans = {}
        for g, lo, hi in runs_all[ci]:
            if g in spans:
                spans[g] = (min(spans[g][0], lo), max(spans[g][1], hi))
            else:
                spans[g] = (lo, hi)
        groups = sorted((g, lo, hi) for g, (lo, hi) in spans.items())
        bounds = sorted({b for _, lo, hi in groups for b in (lo, hi)})
        mms = []
        for a, b in zip(bounds[:-1], bounds[1:]):
            for slot, (g, lo, hi) in enumerate(groups):
                if lo <= a and b <= hi:
                    mms.append((slot, a, b))
        pieces.append({"groups": groups, "mms": mms})

    return in_maps, idx, tp, pieces


def _split16(w):
    ws = np.asarray(w, dtype=np.float32) * WSCALE
    wh = ws.astype(E4_NP)
    wl = (ws - wh.astype(np.float32)).astype(E4_NP)
    return wh, wl


def _lay(a, kc):
    return np.ascontiguousarray(
        np.asarray(a).reshape(kc, 128, -1).transpose(1, 0, 2))


def _add_weights(m, e, W1, W2, W3, Wu1, Wu2, Wu3, wu_cache):
    for nm, w, kc in (("w1", W1[e], 4), ("w2", W2[e], 4), ("w3", W3[e], 2)):
        wh, wl = _split16(w)
        m[nm.replace("w", "wh")] = _lay(wh, kc)
        m[nm.replace("w", "wl")] = _lay(wl, kc)
    if not wu_cache:
        wh, wl = _split16(Wu1)
        wu_cache["wuh1"] = _lay(wh, 4)
        wu_cache["wul1"] = _lay(wl, 4)
        wu_cache["wu2"] = _lay(
            np.asarray(Wu2, dtype=np.float32).astype(BF16_NP), 4)
        wu_cache["wu3"] = _lay(
            np.asarray(Wu3, dtype=np.float32).astype(BF16_NP), 2)
    m.update(wu_cache)


def kernel(u_emb, i_emb, a_emb, o_emb, s,
           W1, b1, W2, b2, W3, b3,
           Wu1, bu1, Wu2, bu2, Wu3, bu3):
    for b in (b1, b2, b3, bu1, bu2, bu3):
        assert not np.any(np.asarray(b)), "kernel assumes zero biases"
    in_maps, idx, tp, pieces = _prepare(u_emb, i_emb, a_emb, o_emb, s)
    wu_cache = {}
    for e in range(R):
        _add_weights(in_maps[e], e, W1, W2, W3, Wu1, Wu2, Wu3, wu_cache)

    nc = _build_kernel(tp, pieces)
    res = run_bass_kernel_spmd(nc, in_maps, core_ids=list(range(N_CORES)))

    s_arr = np.asarray(s)
    out = np.zeros(s_arr.size, dtype=np.float32)
    for e in range(R):
        pred = res.results[e]["pred"].reshape(-1)
        out[idx[e]] = pred[:len(idx[e])] * (1.0 / 256.0)
    return out.reshape(s_arr.shape)
